# revision 1
# baseline (speedup 1.0000x reference)
"""CapsNet Trainium2 kernel: 8-core SPMD Bass/Tile implementation.

Strategy:
  Phase 1 (contraction-parallel): the dominant op is
     dct_emb = relu(norm(log|DCT|) @ W_emb.T + b_emb),  [512,102400]x[102400,768]
  Each core owns a 12800-wide slice of the 102400 contraction dim.
  log|x|+eps is precomputed on host (it is already needed there for the
  global mean/std), and the normalization is affine so it folds into the
  matmul epilogue: (G - mu*s_w)/sigma + b = G/sigma + beta, with W
  pre-divided by sigma and beta = b - mu*s_w/sigma. The per-core partial
  G^T[768,512] products are combined with a single AllReduce.

  Phase 2 is replicated on every core for the full batch (the per-rank
  collective floor on this 8-rank LNC1 topology is ~40-50us, so the two
  batch-mean AllReduces of textbook dynamic routing are far more
  expensive than 8x-redundant compute). Dynamic routing never
  materializes u_hat[B,192,2,64]:
     s_c   = (W2 * c)^T @ u           (contraction over (r,i)=1536)
     P_c   = W3_c^T @ v_c             (back-projected v)
     a_rc  = sum_b sum_i u * P_c      (agreement, mul+reduce on DVE)
  Matmul operands are bf16 (PE streams fp32 at half rate), accumulation
  fp32.
"""

import os
import sys

import numpy as np

if "/opt/trn_rl_repo" not in sys.path:
    sys.path.insert(0, "/opt/trn_rl_repo")

import concourse.bass as bass  # noqa: E402
import concourse.mybir as mybir  # noqa: E402
import concourse.tile as tile  # noqa: E402
from concourse import bacc  # noqa: E402
from concourse.bass_utils import run_bass_kernel_spmd  # noqa: E402
from concourse.masks import make_identity  # noqa: E402

try:
    import ml_dtypes  # noqa: E402

    _BF16 = ml_dtypes.bfloat16
except Exception:  # pragma: no cover
    _BF16 = None

N_CORES = 8
B = 512  # batch (phase 2 works on the full batch)
BT = B // 128  # 4 batch chunks
K, KC = 102400, 12800  # contraction dim, per-core slice
E = 768  # embedding
ET = E // 128  # e chunks
KT = KC // 128  # k tiles per core (100)
GROUP = 5  # k tiles per load DMA
RI = 1536  # (route, in_cap) flat = 192*8
RT = RI // 128  # 12 tiles
NCLS = 2
OC = 64  # out caps channels
F32 = mybir.dt.float32
BF = mybir.dt.bfloat16

PHASE1_DT = os.environ.get("CAPS_P1_DT", "bf16")
# bisection: 1=phase1 only, 2=+AR, 3=+prim/squash/u2, 5=full
STOP = int(os.environ.get("CAPS_STOP", "5"))

_CACHE = {}


def _emit(nc, tc, const, loads, work, ps1, dram, io):
    dt1 = BF if PHASE1_DT == "bf16" else F32
    rg = [list(range(N_CORES))]
    dlog_t, wp, beta, img_t, capt_t, wm2, bias3, w2, w3, y = io

    def debug_out(fill=None):
        out_sb = work.tile([128, 512], F32, tag="outsb", name="outsb")
        nc.vector.memset(out_sb[:], 0.0)
        if fill is not None:
            fill(out_sb)
        for bc in range(BT):
            nc.sync.dma_start(y[bc * 128 : (bc + 1) * 128, :], out_sb[:, :128])

    eps_sq = const.tile([128, 1], F32)
    nc.vector.memset(eps_sq[:], 1e-7)

    # ---------------- phase 1: big matmul ----------------
    g_ps = [ps1.tile([128, B], F32, tag=f"g{ec}", name=f"g{ec}") for ec in range(ET)]
    n_loads = KT // GROUP
    for li in range(n_loads):
        k0 = li * GROUP * 128
        dlog = loads.tile([128, GROUP, B], dt1, tag="dlog")
        nc.sync.dma_start(
            dlog[:],
            dlog_t[k0 : k0 + GROUP * 128, :].rearrange("(s p) b -> p s b", p=128),
        )
        w_tile = loads.tile([128, GROUP, E], dt1, tag="w")
        nc.sync.dma_start(
            w_tile[:],
            wp[k0 : k0 + GROUP * 128, :].rearrange("(s p) e -> p s e", p=128),
        )
        for s in range(GROUP):
            kt = li * GROUP + s
            for ec in range(ET):
                nc.tensor.matmul(
                    g_ps[ec][:],
                    w_tile[:, s, ec * 128 : (ec + 1) * 128],
                    dlog[:, s, :],
                    start=(kt == 0),
                    stop=(kt == KT - 1),
                )

    # evacuate PSUM -> SBUF -> cc_in, AllReduce the partial G^T
    cc_in = dram.tile([E, B], F32)
    for ec in range(ET):
        g_sb = work.tile([128, B], F32, tag="gsb", bufs=1)
        nc.vector.tensor_copy(g_sb[:], g_ps[ec][:])
        nc.sync.dma_start(cc_in[ec * 128 : (ec + 1) * 128, :], g_sb[:])
    if STOP == 1:
        debug_out()
        return

    ar_g = dram.tile([E, B], F32)
    half = (E // 2) * B
    nc.gpsimd.collective_compute(
        "AllReduce",
        mybir.AluOpType.add,
        replica_groups=rg,
        ins=[cc_in[: E // 2, :]],
        outs=[ar_g[: E // 2, :]],
    )
    nc.gpsimd.collective_compute(
        "AllReduce",
        mybir.AluOpType.add,
        replica_groups=rg,
        ins=[cc_in[E // 2 :, :]],
        outs=[ar_g[E // 2 :, :]],
    )
    if STOP == 2:
        debug_out(lambda o: nc.sync.dma_start(o[:, :], ar_g[:128, :]))
        return

    # ---------------- phase 2 constants ----------------
    ident_bf = const.tile([128, 128], BF)
    make_identity(nc, ident_bf[:])
    ident_f = const.tile([128, 128], F32)
    make_identity(nc, ident_f[:])
    beta_sb = const.tile([128, ET], F32)
    nc.sync.dma_start(beta_sb[:], beta[:].rearrange("(t p) -> p t", p=128))
    emb_sb = {}  # (m, et) -> [128, B] bf16 tile (lhsT source for prim)
    for m, src in ((0, img_t), (1, capt_t)):
        for et in range(ET):
            t = const.tile([128, B], BF, tag=f"emb{m}_{et}", name=f"emb{m}_{et}")
            nc.sync.dma_start(t[:], src[et * 128 : (et + 1) * 128, :])
            emb_sb[(m, et)] = t
    wm2_sb = {}
    for m in range(3):
        for et in range(ET):
            t = const.tile([128, 512], BF, tag=f"wm2_{m}_{et}", name=f"wm2_{m}_{et}")
            nc.sync.dma_start(t[:], wm2[m, et * 128 : (et + 1) * 128, :])
            wm2_sb[(m, et)] = t
    bias_sb = []
    for m in range(3):
        t = const.tile([1, 512], BF, tag=f"bias{m}", name=f"bias{m}")
        nc.sync.dma_start(t[:], bias3[m : m + 1, :])
        bias_sb.append(t)
    ones1 = const.tile([1, 128], BF)
    nc.vector.memset(ones1[:], 1.0)
    w2_sb = []
    for t_ in range(RT):
        t = const.tile([128, 128], BF, tag=f"w2_{t_}", name=f"w2_{t_}")
        nc.sync.dma_start(t[:], w2[t_ * 128 : (t_ + 1) * 128, :])
        w2_sb.append(t)
    w3_sb = []
    for c in range(NCLS):
        t = const.tile([OC, RI], BF, tag=f"w3_{c}", name=f"w3_{c}")
        nc.sync.dma_start(t[:], w3[c * OC : (c + 1) * OC, :])
        w3_sb.append(t)

    # dct embedding tiles: relu(ar_g + beta) -> bf16
    for et in range(ET):
        gp = work.tile([128, B], F32, tag="gp")
        nc.sync.dma_start(gp[:], ar_g[et * 128 : (et + 1) * 128, :])
        t = const.tile([128, B], BF, tag=f"emb2_{et}", name=f"emb2_{et}")
        nc.scalar.activation(
            t[:],
            gp[:],
            mybir.ActivationFunctionType.Relu,
            bias=beta_sb[:, et : et + 1],
        )
        emb_sb[(2, et)] = t

    # ---------------- primary caps + squash + transpose to u2 ----------------
    # per (modality, b-chunk) chains; img/capt emitted first so they
    # execute under the AllReduce, dct chains pipeline behind it.
    u2_all = const.tile([128, RT, B], BF)  # [(r,i)-tile, t, b]

    def prim_chain(m, bc):
        pm = ps1.tile([128, 512], F32, tag="pp", bufs=2, name="pm")
        for et in range(ET):
            nc.tensor.matmul(
                pm[:],
                emb_sb[(m, et)][:, bc * 128 : (bc + 1) * 128],
                wm2_sb[(m, et)][:],
                start=(et == 0),
                stop=False,
            )
        nc.tensor.matmul(pm[:], ones1[:], bias_sb[m][:], start=False, stop=True)
        upre_c = work.tile([128, 512], F32, tag="upre", bufs=3, name="upre")
        nc.scalar.copy(upre_c[:], pm[:])
        # squash over i (groups of 8 in the free dim), 64 routes here
        sq8 = work.tile([128, 512], F32, tag="sq8", bufs=2)
        nc.vector.tensor_mul(sq8[:], upre_c[:], upre_c[:])
        usq = work.tile([128, 64], F32, tag="usq")
        nc.vector.tensor_reduce(
            usq[:],
            sq8[:].rearrange("p (r i) -> p r i", i=8),
            axis=mybir.AxisListType.X,
            op=mybir.AluOpType.add,
        )
        t1 = work.tile([128, 64], F32, tag="fa")
        nc.scalar.activation(
            t1[:], usq[:], mybir.ActivationFunctionType.Sqrt, bias=eps_sq[:]
        )
        t2 = work.tile([128, 64], F32, tag="fb")
        nc.vector.tensor_scalar_add(t2[:], usq[:], 1.0)
        t3 = work.tile([128, 64], F32, tag="fc")
        nc.vector.tensor_mul(t3[:], t1[:], t2[:])
        t4 = work.tile([128, 64], F32, tag="fd")
        nc.vector.reciprocal(t4[:], t3[:])
        t5 = work.tile([128, 64], F32, tag="fe")
        nc.vector.tensor_mul(t5[:], t4[:], usq[:])
        usquash = work.tile([128, 512], BF, tag="usquash", bufs=2)
        nc.vector.tensor_tensor(
            usquash[:].rearrange("p (r i) -> p r i", i=8),
            upre_c[:].rearrange("p (r i) -> p r i", i=8),
            t5[:].broadcast_to([128, 64, 8]),
            op=mybir.AluOpType.mult,
        )
        for j in range(4):
            tp = ps1.tile([128, 128], BF, tag="pp", bufs=2, name="tp")
            nc.tensor.transpose(
                tp[:], usquash[:, j * 128 : (j + 1) * 128], ident_bf[:]
            )
            nc.vector.tensor_copy(
                u2_all[:, 4 * m + j, bc * 128 : (bc + 1) * 128], tp[:]
            )

    for m in (0, 1):
        for bc in range(BT):
            prim_chain(m, bc)
    for bc in range(BT):
        prim_chain(2, bc)
    if STOP == 3:
        debug_out(lambda o: nc.vector.tensor_copy(o[:, :], u2_all[:, 0, :]))
        return

    # ---------------- dynamic routing (replicated, no collectives) ----------
    a_d = [dram.tile([NCLS, 192], F32, name=f"a_d{i}") for i in range(2)]
    c_dram = [dram.tile([192, NCLS], F32, name=f"c_dram{i}") for i in range(2)]
    b_cur = None  # [2,192] logits tile

    v_both = None
    for it in range(3):
        rnd = it - 1
        if it == 0:
            mset = w2_sb  # uniform c folded into evac scale 1/192
        else:
            # softmax(b_cur) over routes -> c_sm [2,192]
            mx = work.tile([NCLS, 1], F32, tag="smx", name="smx")
            nc.vector.tensor_reduce(
                mx[:], b_cur[:], axis=mybir.AxisListType.X, op=mybir.AluOpType.max
            )
            mxn = work.tile([NCLS, 1], F32, tag="smxn", name="smxn")
            nc.vector.tensor_scalar_mul(mxn[:], mx[:], -1.0)
            ex = work.tile([NCLS, 192], F32, tag="sex", name="sex")
            nc.scalar.activation(
                ex[:], b_cur[:], mybir.ActivationFunctionType.Exp, bias=mxn[:]
            )
            sm = work.tile([NCLS, 1], F32, tag="ssm", name="ssm")
            nc.vector.tensor_reduce(
                sm[:], ex[:], axis=mybir.AxisListType.X, op=mybir.AluOpType.add
            )
            rcp = work.tile([NCLS, 1], F32, tag="srcp", name="srcp")
            nc.vector.reciprocal(rcp[:], sm[:])
            c_sm = work.tile([NCLS, 192], F32, tag="scs", name="scs")
            nc.vector.tensor_scalar(
                c_sm[:], ex[:], rcp[:], None, op0=mybir.AluOpType.mult
            )
            # c [2,192] -> DRAM [192,2] -> broadcast-read c_exp [128, RT, 2]
            nc.sync.dma_start(c_dram[rnd][:].rearrange("r c -> c r"), c_sm[:])
            c_exp = work.tile([128, RT, NCLS], F32, tag="cexp", name="cexp")
            for t_ in range(RT):
                nc.sync.dma_start(
                    c_exp[:, t_, :],
                    c_dram[rnd][16 * t_ : 16 * (t_ + 1), :]
                    .broadcast_to([16, NCLS, 8])
                    .rearrange("j c r -> j r c"),
                )
            mset = []
            for t_ in range(RT):
                pair = []
                for c in range(NCLS):
                    msc = work.tile(
                        [128, OC], BF, tag=f"msc{c}", bufs=12, name=f"msc{c}"
                    )
                    nc.vector.tensor_scalar(
                        msc[:],
                        w2_sb[t_][:, c * OC : (c + 1) * OC],
                        c_exp[:, t_, c : c + 1],
                        None,
                        op0=mybir.AluOpType.mult,
                    )
                    pair.append(msc)
                mset.append(pair)

        # s per class (plain matmuls; tile_position drops fresh-lhsT deps)
        v_both = []
        for c in range(NCLS):
            s_ps = ps1.tile([OC, B], F32, tag="pp", bufs=2, name="s_ps")
            for t_ in range(RT):
                lh = (
                    mset[t_][:, c * OC : (c + 1) * OC]
                    if it == 0
                    else mset[t_][c][:]
                )
                nc.tensor.matmul(
                    s_ps[:],
                    lh,
                    u2_all[:, t_, :],
                    start=(t_ == 0),
                    stop=(t_ == RT - 1),
                )
            s_sb = work.tile([OC, B], F32, tag=f"ssb{c}", name=f"ssb{c}")
            nc.scalar.mul(s_sb[:], s_ps[:], (1.0 / 192.0) if it == 0 else 1.0)
            # elementwise digit squash on [OC, B]
            sq = work.tile([OC, B], F32, tag="dsq", name="dsq")
            nc.vector.tensor_mul(sq[:], s_sb[:], s_sb[:])
            d1 = work.tile([OC, B], F32, tag="dd1", name="dd1")
            nc.scalar.activation(
                d1[:], sq[:], mybir.ActivationFunctionType.Sqrt, bias=eps_sq[:OC, :]
            )
            d2 = work.tile([OC, B], F32, tag="dd2", name="dd2")
            nc.vector.tensor_scalar_add(d2[:], sq[:], 1.0)
            d3 = work.tile([OC, B], F32, tag="dd3", name="dd3")
            nc.vector.tensor_mul(d3[:], d1[:], d2[:])
            d4 = work.tile([OC, B], F32, tag="dd4", name="dd4")
            nc.vector.reciprocal(d4[:], d3[:])
            d5 = work.tile([OC, B], F32, tag="dd5", name="dd5")
            nc.vector.tensor_mul(d5[:], d4[:], sq[:])
            vv = work.tile([OC, B], F32, tag=f"vb{c}", name=f"vb{c}", bufs=2)
            nc.vector.tensor_mul(vv[:], d5[:], s_sb[:])
            v_both.append(vv)

        if it < 2:
            # agreement: abar[r,c] = sum_b sum_i u2 * (W3_c^T @ v_c)
            for c in range(NCLS):
                v_bf = work.tile([OC, B], BF, tag=f"vbf{c}", name=f"vbf{c}")
                nc.scalar.copy(v_bf[:], v_both[c][:])
                dcat = work.tile([128, RT], F32, tag=f"dcat{c}", name=f"dcat{c}")
                for t_ in range(RT):
                    pc = ps1.tile([128, B], F32, tag="pp", bufs=2, name="pc")
                    nc.tensor.matmul(
                        pc[:],
                        w3_sb[c][:, t_ * 128 : (t_ + 1) * 128],
                        v_bf[:],
                        start=True,
                        stop=True,
                    )
                    prod = work.tile([128, B], BF, tag="prod", name="prod")
                    nc.vector.tensor_mul(prod[:], u2_all[:, t_, :], pc[:])
                    nc.scalar.activation(
                        prod[:],
                        prod[:],
                        mybir.ActivationFunctionType.Copy,
                        accum_out=dcat[:, t_ : t_ + 1],
                    )
                dtp = ps1.tile([RT, 128], F32, tag="pp", bufs=2, name="dtp")
                nc.tensor.transpose(dtp[:], dcat[:], ident_f[:])
                abar = work.tile([RT, 16], F32, tag=f"abar{c}", name=f"abar{c}")
                nc.vector.tensor_reduce(
                    abar[:],
                    dtp[:].rearrange("p (r i) -> p r i", i=8),
                    axis=mybir.AxisListType.X,
                    op=mybir.AluOpType.add,
                )
                nc.sync.dma_start(
                    a_d[it][c, :].rearrange("(t j) -> t j", t=RT), abar[:]
                )
            ld = work.tile([NCLS, 192], F32, tag=f"arld{it}", name=f"arld{it}")
            nc.sync.dma_start(ld[:], a_d[it][:])
            b_new = work.tile([NCLS, 192], F32, tag=f"bcur{it}", name=f"bcur{it}")
            if it == 0:
                nc.scalar.mul(b_new[:], ld[:], 1.0 / B)
            else:
                scaled = work.tile([NCLS, 192], F32, tag="arsc", name="arsc")
                nc.vector.tensor_scalar_mul(scaled[:], ld[:], 1.0 / B)
                nc.vector.tensor_add(b_new[:], b_cur[:], scaled[:])
            b_cur = b_new

    # final output: y[b, (c,o)] via PE transposes of v_c
    for bc in range(BT):
        ob = work.tile([128, 128], F32, tag="ob", bufs=2, name="ob")
        for c in range(NCLS):
            vt = ps1.tile([128, OC], F32, tag="pp", bufs=2, name="vt")
            nc.tensor.transpose(
                vt[:],
                v_both[c][:, bc * 128 : (bc + 1) * 128],
                ident_f[:OC, :OC],
            )
            nc.vector.tensor_copy(ob[:, c * OC : (c + 1) * OC], vt[:])
        nc.sync.dma_start(y[bc * 128 : (bc + 1) * 128, :], ob[:])


def _build_program():
    dt1 = BF if PHASE1_DT == "bf16" else F32
    nc = bacc.Bacc(num_devices=N_CORES)

    dlog_t = nc.declare_dram_parameter("dlog_t", [KC, B], dt1, isOutput=False)
    wp = nc.declare_dram_parameter("wp", [KC, E], dt1, isOutput=False)
    beta = nc.declare_dram_parameter("beta", [E], F32, isOutput=False)
    img_t = nc.declare_dram_parameter("img_t", [E, B], BF, isOutput=False)
    capt_t = nc.declare_dram_parameter("capt_t", [E, B], BF, isOutput=False)
    wm2 = nc.declare_dram_parameter("wm2", [3, E, 512], BF, isOutput=False)
    bias3 = nc.declare_dram_parameter("bias3", [3, 512], BF, isOutput=False)
    w2 = nc.declare_dram_parameter("w2", [RI, 128], BF, isOutput=False)
    w3 = nc.declare_dram_parameter("w3", [128, RI], BF, isOutput=False)
    y = nc.declare_dram_parameter("y", [B, 128], F32, isOutput=True)
    io = (dlog_t, wp, beta, img_t, capt_t, wm2, bias3, w2, w3, y)

    with tile.TileContext(nc) as tc:
        with (
            tc.tile_pool(name="const", bufs=1) as const,
            tc.tile_pool(name="loads", bufs=3) as loads,
            tc.tile_pool(name="work", bufs=2) as work,
            tc.tile_pool(name="ps1", bufs=1, space="PSUM") as ps1,
            tc.tile_pool(name="dram", bufs=1, space="DRAM") as dram,
        ):
            _emit(nc, tc, const, loads, work, ps1, dram, io)

    nc.compile()
    return nc


def _host_prep(inputs):
    """Numpy-side sharding/layout prep. Returns per-core input maps."""
    img_emb = np.asarray(inputs["img_emb"], dtype=np.float32)
    capt_emb = np.asarray(inputs["capt_emb"], dtype=np.float32)
    dct = np.asarray(inputs["DCT_features"], dtype=np.float32).reshape(B, K)
    w_emb = np.asarray(inputs["W_emb"], dtype=np.float32)
    b_emb = np.asarray(inputs["b_emb"], dtype=np.float32)
    w_digit = np.asarray(inputs["W_digit"], dtype=np.float32)

    dlog = np.log(np.abs(dct) + 1e-12)
    mu = float(dlog.mean(dtype=np.float64))
    sigma = float(dlog.std(ddof=1, dtype=np.float64))
    s_w = w_emb.sum(axis=1, dtype=np.float64)
    beta = (b_emb - (mu / sigma) * s_w).astype(np.float32)

    np_dt1 = _BF16 if PHASE1_DT == "bf16" else np.float32
    dlog_T = np.ascontiguousarray(dlog.T).astype(np_dt1)  # [K, B]
    wp = np.ascontiguousarray(w_emb.T / sigma).astype(np_dt1)  # [K, E]

    wm2 = np.stack(
        [
            np.ascontiguousarray(
                np.asarray(inputs[f"W_{m}"], dtype=np.float32).transpose(2, 1, 0)
            ).reshape(E, 512)
            for m in ("img", "capt", "dct")
        ]
    ).astype(_BF16)  # [3, E, 512]
    bias3 = np.stack(
        [
            np.ascontiguousarray(
                np.asarray(inputs[f"b_{m}"], dtype=np.float32).T
            ).reshape(512)
            for m in ("img", "capt", "dct")
        ]
    ).astype(_BF16)  # [3, 512]
    w2 = (
        np.ascontiguousarray(w_digit.transpose(0, 3, 1, 2))
        .reshape(RI, 128)
        .astype(_BF16)
    )
    w3 = np.concatenate(
        [
            np.ascontiguousarray(w_digit[:, c].transpose(1, 0, 2)).reshape(OC, RI)
            for c in range(NCLS)
        ]
    ).astype(_BF16)  # [128, RI]
    img_T = np.ascontiguousarray(img_emb.T).astype(_BF16)  # [E, B]
    capt_T = np.ascontiguousarray(capt_emb.T).astype(_BF16)

    in_maps = []
    for c in range(N_CORES):
        in_maps.append(
            {
                "dlog_t": np.ascontiguousarray(dlog_T[c * KC : (c + 1) * KC]),
                "wp": np.ascontiguousarray(wp[c * KC : (c + 1) * KC]),
                "beta": beta,
                "img_t": img_T,
                "capt_t": capt_T,
                "wm2": wm2,
                "bias3": bias3,
                "w2": w2,
                "w3": w3,
            }
        )
    return in_maps


def kernel(**inputs) -> np.ndarray:
    if "nc" not in _CACHE:
        _CACHE["nc"] = _build_program()
    nc = _CACHE["nc"]
    in_maps = _host_prep(inputs)
    trace = bool(int(os.environ.get("CAPS_TRACE", "0")))
    res = run_bass_kernel_spmd(nc, in_maps, list(range(N_CORES)), trace=trace)
    _CACHE["last_result"] = res
    out = res.results[0]["y"].reshape(B, NCLS, OC)
    return np.ascontiguousarray(out)[:, :, :, None]



# revision 6
# speedup vs baseline: 1.2528x; 1.2528x over previous
"""CapsNet Trainium2 kernel: 8-core SPMD Bass/Tile implementation.

Strategy (v2):
  Phase 1 (contraction-parallel): the dominant op is
     dct_emb = relu(norm(log|DCT|) @ W_emb.T + b_emb),  [512,102400]x[102400,768]
  Each core owns a 12800-wide slice of the 102400 contraction dim
  (~128us of bf16 PE time — the roofline). log|x| is precomputed on
  host (needed there for the global mean/std anyway) and the affine
  normalization folds into the matmul epilogue.

  Phase 2 (batch-parallel): the per-core partial G^T[768,512] products
  are combined with a ReduceScatter over the batch dim (DRAM layout
  [8, 768, 64] so rank r receives exactly its 64 batch columns). Each
  core then runs primary caps + dynamic routing for its own 64 rows.
  The two batch-mean agreement reductions of dynamic routing are tiny
  [2,192] AllReduces (~10us floor each on the 8-core intra-chip mesh).
  The host concatenates the per-core [64,128] outputs.

  Dynamic routing never materializes u_hat[B,192,2,64]:
     s_c   = (W2 * c)^T @ u           (contraction over (r,i)=1536)
     P_c   = W3_c^T @ v_c             (back-projected v)
     a_rc  = sum_b sum_i u * P_c      (agreement, mul+reduce)
  Matmul operands are bf16 (PE streams fp32 at half rate), accumulation
  fp32.
"""

import os
import sys

import numpy as np

if "/opt/trn_rl_repo" not in sys.path:
    sys.path.insert(0, "/opt/trn_rl_repo")

import concourse.bass as bass  # noqa: E402
import concourse.mybir as mybir  # noqa: E402
import concourse.tile as tile  # noqa: E402
from concourse import bacc  # noqa: E402
from concourse.bass_utils import run_bass_kernel_spmd  # noqa: E402
from concourse.masks import make_identity  # noqa: E402

try:
    import ml_dtypes  # noqa: E402

    _BF16 = ml_dtypes.bfloat16
except Exception:  # pragma: no cover
    _BF16 = None

N_CORES = 8
B = 512  # global batch
BL = B // N_CORES  # 64 local batch rows per core
K, KC = 102400, 12800  # contraction dim, per-core slice
E = 768  # embedding
ET = E // 128  # e chunks
KT = KC // 128  # k tiles per core (100)
GROUP = 5  # k tiles per load DMA
RI = 1536  # (route, in_cap) flat = 192*8
RT = RI // 128  # 12 tiles
NCLS = 2
OC = 64  # out caps channels
F32 = mybir.dt.float32
BF = mybir.dt.bfloat16

PHASE1_DT = os.environ.get("CAPS_P1_DT", "bf16")
# bisection: 1=phase1 only, 2=+RS, 3=+prim/squash/u2, 5=full
STOP = int(os.environ.get("CAPS_STOP", "5"))

_CACHE = {}


def _emit(nc, tc, const, loads, work, ps1, dram, io):
    dt1 = BF if PHASE1_DT == "bf16" else F32
    rg = [list(range(N_CORES))]
    dlog_t, wp, beta, img_t, capt_t, wm2, bias3, w2, w3, y = io

    def debug_out(fill=None):
        out_sb = work.tile([BL, 128], F32, tag="outsb", name="outsb")
        nc.vector.memset(out_sb[:], 0.0)
        if fill is not None:
            fill(out_sb)
        nc.sync.dma_start(y[:, :], out_sb[:])

    eps_sq = const.tile([128, 1], F32)
    nc.vector.memset(eps_sq[:], 1e-7)

    # ---------------- phase 2 constants (loads issue early, free DMA) ----
    ident_bf = const.tile([128, 128], BF)
    make_identity(nc, ident_bf[:])
    ident_f = const.tile([128, 128], F32)
    make_identity(nc, ident_f[:])
    beta_sb = const.tile([128, ET], F32)
    nc.sync.dma_start(beta_sb[:], beta[:].rearrange("(t p) -> p t", p=128))
    emb_sb = {}  # (m, et) -> [128, BL] bf16 tile (lhsT source for prim)
    for m, src in ((0, img_t), (1, capt_t)):
        for et in range(ET):
            t = const.tile([128, BL], BF, tag=f"emb{m}_{et}", name=f"emb{m}_{et}")
            nc.sync.dma_start(t[:], src[et * 128 : (et + 1) * 128, :])
            emb_sb[(m, et)] = t
    wm2_sb = {}
    for m in range(3):
        for et in range(ET):
            t = const.tile([128, 512], BF, tag=f"wm2_{m}_{et}", name=f"wm2_{m}_{et}")
            nc.sync.dma_start(t[:], wm2[m, et * 128 : (et + 1) * 128, :])
            wm2_sb[(m, et)] = t
    bias_sb = []
    for m in range(3):
        t = const.tile([1, 512], BF, tag=f"bias{m}", name=f"bias{m}")
        nc.sync.dma_start(t[:], bias3[m : m + 1, :])
        bias_sb.append(t)
    ones1 = const.tile([1, BL], BF)
    nc.vector.memset(ones1[:], 1.0)
    w2_sb = []
    for t_ in range(RT):
        t = const.tile([128, 128], BF, tag=f"w2_{t_}", name=f"w2_{t_}")
        nc.sync.dma_start(t[:], w2[t_ * 128 : (t_ + 1) * 128, :])
        w2_sb.append(t)
    w3_sb = []
    for c in range(NCLS):
        t = const.tile([OC, RI], BF, tag=f"w3_{c}", name=f"w3_{c}")
        nc.sync.dma_start(t[:], w3[c * OC : (c + 1) * OC, :])
        w3_sb.append(t)

    # ---------------- phase 1: big matmul ----------------
    g_ps = [ps1.tile([128, B], F32, tag=f"g{ec}", name=f"g{ec}") for ec in range(ET)]
    n_loads = KT // GROUP
    for li in range(n_loads):
        k0 = li * GROUP * 128
        dlog = loads.tile([128, GROUP, B], dt1, tag="dlog")
        nc.sync.dma_start(
            dlog[:],
            dlog_t[k0 : k0 + GROUP * 128, :].rearrange("(s p) b -> p s b", p=128),
        )
        w_tile = loads.tile([128, GROUP, E], dt1, tag="w")
        nc.sync.dma_start(
            w_tile[:],
            wp[k0 : k0 + GROUP * 128, :].rearrange("(s p) e -> p s e", p=128),
        )
        for s in range(GROUP):
            kt = li * GROUP + s
            for ec in range(ET):
                nc.tensor.matmul(
                    g_ps[ec][:],
                    w_tile[:, s, ec * 128 : (ec + 1) * 128],
                    dlog[:, s, :],
                    start=(kt == 0),
                    stop=(kt == KT - 1),
                )

    # evacuate PSUM -> SBUF -> cc_in[8, 768, 64]; ReduceScatter over batch
    cc_in = dram.tile([N_CORES, E, BL], F32)
    for ec in range(ET):
        g_sb = work.tile([128, B], F32, tag="gsb", bufs=2)
        if ec % 2 == 0:
            nc.vector.tensor_copy(g_sb[:], g_ps[ec][:])
        else:
            nc.scalar.copy(g_sb[:], g_ps[ec][:])
        nc.sync.dma_start(
            cc_in[:, ec * 128 : (ec + 1) * 128, :].rearrange("r p b -> p r b"),
            g_sb[:].rearrange("p (r b) -> p r b", r=N_CORES),
        )
    if STOP == 1:
        debug_out()
        return

    rs_out = dram.tile([E, BL], F32)
    nc.gpsimd.collective_compute(
        "ReduceScatter",
        mybir.AluOpType.add,
        replica_groups=rg,
        ins=[cc_in[:]],
        outs=[rs_out[:]],
    )
    if STOP == 2:
        debug_out(lambda o: nc.sync.dma_start(o[:, :], rs_out[:BL, :128]))
        return

    # ---------------- primary caps + squash + transpose to u2 ----------
    u2_all = const.tile([128, RT, BL], BF)  # [(r,i)-tile, t, b]

    def prim_chain(m):
        pm = ps1.tile([BL, 512], F32, tag="pp", bufs=2, name="pm")
        for et in range(ET):
            nc.tensor.matmul(
                pm[:],
                emb_sb[(m, et)][:],
                wm2_sb[(m, et)][:],
                start=(et == 0),
                stop=False,
            )
        nc.tensor.matmul(pm[:], ones1[:], bias_sb[m][:], start=False, stop=True)
        upre_c = work.tile([BL, 512], F32, tag="upre", bufs=3, name="upre")
        nc.scalar.copy(upre_c[:], pm[:])
        # squash over i (groups of 8 in the free dim), 64 routes here
        sq8 = work.tile([BL, 512], F32, tag="sq8", bufs=2)
        nc.vector.tensor_mul(sq8[:], upre_c[:], upre_c[:])
        usq = work.tile([BL, 64], F32, tag="usq")
        nc.vector.tensor_reduce(
            usq[:],
            sq8[:].rearrange("p (r i) -> p r i", i=8),
            axis=mybir.AxisListType.X,
            op=mybir.AluOpType.add,
        )
        t1 = work.tile([BL, 64], F32, tag="fa")
        nc.scalar.activation(
            t1[:], usq[:], mybir.ActivationFunctionType.Sqrt, bias=eps_sq[:BL, :]
        )
        t2 = work.tile([BL, 64], F32, tag="fb")
        nc.vector.tensor_scalar_add(t2[:], usq[:], 1.0)
        t3 = work.tile([BL, 64], F32, tag="fc")
        nc.vector.tensor_mul(t3[:], t1[:], t2[:])
        t4 = work.tile([BL, 64], F32, tag="fd")
        nc.vector.reciprocal(t4[:], t3[:])
        t5 = work.tile([BL, 64], F32, tag="fe")
        nc.vector.tensor_mul(t5[:], t4[:], usq[:])
        usquash = work.tile([BL, 512], BF, tag="usquash", bufs=2)
        nc.vector.tensor_tensor(
            usquash[:].rearrange("p (r i) -> p r i", i=8),
            upre_c[:].rearrange("p (r i) -> p r i", i=8),
            t5[:].broadcast_to([BL, 64, 8]),
            op=mybir.AluOpType.mult,
        )
        for j in range(4):
            tp = ps1.tile([128, BL], BF, tag="pp", bufs=2, name="tp")
            nc.tensor.transpose(
                tp[:], usquash[:, j * 128 : (j + 1) * 128], ident_bf[:BL, :BL]
            )
            nc.vector.tensor_copy(u2_all[:, 4 * m + j, :], tp[:])

    # img/capt chains execute on the PE during the ReduceScatter
    prim_chain(0)
    prim_chain(1)

    # dct embedding tiles: relu(rs_out + beta) -> bf16
    gp = work.tile([128, ET, BL], F32, tag="gp")
    nc.sync.dma_start(gp[:], rs_out[:].rearrange("(t p) b -> p t b", p=128))
    for et in range(ET):
        t = const.tile([128, BL], BF, tag=f"emb2_{et}", name=f"emb2_{et}")
        nc.scalar.activation(
            t[:],
            gp[:, et, :],
            mybir.ActivationFunctionType.Relu,
            bias=beta_sb[:, et : et + 1],
        )
        emb_sb[(2, et)] = t
    prim_chain(2)
    if STOP == 3:
        debug_out(lambda o: nc.vector.tensor_copy(o[:, :], u2_all[:BL, 0, :].broadcast_to([BL, 128])))
        return

    # ---------------- dynamic routing (batch-sharded) -------------------
    a_d = [dram.tile([NCLS, 192], F32, name=f"a_d{i}") for i in range(2)]
    ar_a = [
        dram.tile([NCLS, 192], F32, addr_space="Shared", name=f"ar_a{i}")
        for i in range(2)
    ]
    c_dram = [dram.tile([192, NCLS], F32, name=f"c_dram{i}") for i in range(2)]
    b_cur = None  # [2,192] logits tile

    v_both = None
    for it in range(3):
        rnd = it - 1
        if it == 0:
            mset = w2_sb  # uniform c folded into evac scale 1/192
        else:
            # softmax(b_cur) over routes -> c_sm [2,192]
            mx = work.tile([NCLS, 1], F32, tag="smx", name="smx")
            nc.vector.tensor_reduce(
                mx[:], b_cur[:], axis=mybir.AxisListType.X, op=mybir.AluOpType.max
            )
            mxn = work.tile([NCLS, 1], F32, tag="smxn", name="smxn")
            nc.vector.tensor_scalar_mul(mxn[:], mx[:], -1.0)
            ex = work.tile([NCLS, 192], F32, tag="sex", name="sex")
            nc.scalar.activation(
                ex[:], b_cur[:], mybir.ActivationFunctionType.Exp, bias=mxn[:]
            )
            sm = work.tile([NCLS, 1], F32, tag="ssm", name="ssm")
            nc.vector.tensor_reduce(
                sm[:], ex[:], axis=mybir.AxisListType.X, op=mybir.AluOpType.add
            )
            rcp = work.tile([NCLS, 1], F32, tag="srcp", name="srcp")
            nc.vector.reciprocal(rcp[:], sm[:])
            c_sm = work.tile([NCLS, 192], F32, tag="scs", name="scs")
            nc.vector.tensor_scalar(
                c_sm[:], ex[:], rcp[:], None, op0=mybir.AluOpType.mult
            )
            # c [2,192] -> DRAM [192,2] -> broadcast-read c_exp [128, RT, 2]
            nc.sync.dma_start(c_dram[rnd][:].rearrange("r c -> c r"), c_sm[:])
            c_exp = work.tile([128, RT, NCLS], F32, tag="cexp", name="cexp")
            for t_ in range(RT):
                nc.sync.dma_start(
                    c_exp[:, t_, :],
                    c_dram[rnd][16 * t_ : 16 * (t_ + 1), :]
                    .broadcast_to([16, NCLS, 8])
                    .rearrange("j c r -> j r c"),
                )
            mset = []
            for t_ in range(RT):
                pair = []
                for c in range(NCLS):
                    msc = work.tile(
                        [128, OC], BF, tag=f"msc{c}", bufs=12, name=f"msc{c}"
                    )
                    if (t_ + c) % 2 == 0:
                        nc.vector.tensor_scalar(
                            msc[:],
                            w2_sb[t_][:, c * OC : (c + 1) * OC],
                            c_exp[:, t_, c : c + 1],
                            None,
                            op0=mybir.AluOpType.mult,
                        )
                    else:
                        nc.scalar.activation(
                            msc[:],
                            w2_sb[t_][:, c * OC : (c + 1) * OC],
                            mybir.ActivationFunctionType.Copy,
                            scale=c_exp[:, t_, c : c + 1],
                        )
                    pair.append(msc)
                mset.append(pair)

        # s per class
        v_both = []
        for c in range(NCLS):
            s_ps = ps1.tile([OC, BL], F32, tag="pp", bufs=2, name="s_ps")
            for t_ in range(RT):
                lh = (
                    mset[t_][:, c * OC : (c + 1) * OC]
                    if it == 0
                    else mset[t_][c][:]
                )
                nc.tensor.matmul(
                    s_ps[:],
                    lh,
                    u2_all[:, t_, :],
                    start=(t_ == 0),
                    stop=(t_ == RT - 1),
                )
            s_sb = work.tile([OC, BL], F32, tag=f"ssb{c}", name=f"ssb{c}")
            nc.scalar.mul(s_sb[:], s_ps[:], (1.0 / 192.0) if it == 0 else 1.0)
            # elementwise digit squash on [OC, BL]
            sq = work.tile([OC, BL], F32, tag=f"dsq{c}", name=f"dsq{c}")
            nc.vector.tensor_mul(sq[:], s_sb[:], s_sb[:])
            d1 = work.tile([OC, BL], F32, tag=f"dd1{c}", name=f"dd1{c}")
            nc.scalar.activation(
                d1[:], sq[:], mybir.ActivationFunctionType.Sqrt, bias=eps_sq[:OC, :]
            )
            d2 = work.tile([OC, BL], F32, tag=f"dd2{c}", name=f"dd2{c}")
            nc.vector.tensor_scalar_add(d2[:], sq[:], 1.0)
            d3 = work.tile([OC, BL], F32, tag=f"dd3{c}", name=f"dd3{c}")
            nc.vector.tensor_mul(d3[:], d1[:], d2[:])
            d4 = work.tile([OC, BL], F32, tag=f"dd4{c}", name=f"dd4{c}")
            nc.vector.reciprocal(d4[:], d3[:])
            d5 = work.tile([OC, BL], F32, tag=f"dd5{c}", name=f"dd5{c}")
            nc.vector.tensor_mul(d5[:], d4[:], sq[:])
            vv = work.tile([OC, BL], F32, tag=f"vb{c}", name=f"vb{c}", bufs=2)
            nc.vector.tensor_mul(vv[:], d5[:], s_sb[:])
            v_both.append(vv)

        if it < 2:
            # agreement: abar[r,c] = sum_b sum_i u2 * (W3_c^T @ v_c)
            for c in range(NCLS):
                v_bf = work.tile([OC, BL], BF, tag=f"vbf{c}", name=f"vbf{c}")
                nc.scalar.copy(v_bf[:], v_both[c][:])
                dcat = work.tile([128, RT], F32, tag=f"dcat{c}", name=f"dcat{c}")
                for t_ in range(RT):
                    pc = ps1.tile([128, BL], F32, tag="pp", bufs=2, name="pc")
                    nc.tensor.matmul(
                        pc[:],
                        w3_sb[c][:, t_ * 128 : (t_ + 1) * 128],
                        v_bf[:],
                        start=True,
                        stop=True,
                    )
                    prod = work.tile([128, BL], BF, tag="prod", name="prod")
                    nc.vector.tensor_mul(prod[:], u2_all[:, t_, :], pc[:])
                    nc.scalar.activation(
                        prod[:],
                        prod[:],
                        mybir.ActivationFunctionType.Copy,
                        accum_out=dcat[:, t_ : t_ + 1],
                    )
                dtp = ps1.tile([RT, 128], F32, tag="pp", bufs=2, name="dtp")
                nc.tensor.transpose(dtp[:], dcat[:], ident_f[:])
                abar = work.tile([RT, 16], F32, tag=f"abar{c}", name=f"abar{c}")
                nc.vector.tensor_reduce(
                    abar[:],
                    dtp[:].rearrange("p (r i) -> p r i", i=8),
                    axis=mybir.AxisListType.X,
                    op=mybir.AluOpType.add,
                )
                nc.sync.dma_start(
                    a_d[it][c, :].rearrange("(t j) -> t j", t=RT), abar[:]
                )
            nc.gpsimd.collective_compute(
                "AllReduce",
                mybir.AluOpType.add,
                replica_groups=rg,
                ins=[a_d[it][:]],
                outs=[ar_a[it][:]],
            )
            ld = work.tile([NCLS, 192], F32, tag=f"arld{it}", name=f"arld{it}")
            nc.sync.dma_start(ld[:], ar_a[it][:])
            b_new = work.tile([NCLS, 192], F32, tag=f"bcur{it}", name=f"bcur{it}")
            if it == 0:
                nc.scalar.mul(b_new[:], ld[:], 1.0 / B)
            else:
                scaled = work.tile([NCLS, 192], F32, tag="arsc", name="arsc")
                nc.vector.tensor_scalar_mul(scaled[:], ld[:], 1.0 / B)
                nc.vector.tensor_add(b_new[:], b_cur[:], scaled[:])
            b_cur = b_new

    # final output: y[b, (c,o)] via PE transposes of v_c
    ob = work.tile([BL, 128], F32, tag="ob", bufs=2, name="ob")
    for c in range(NCLS):
        vt = ps1.tile([BL, OC], F32, tag="pp", bufs=2, name="vt")
        nc.tensor.transpose(vt[:], v_both[c][:], ident_f[:OC, :OC])
        nc.vector.tensor_copy(ob[:, c * OC : (c + 1) * OC], vt[:])
    nc.sync.dma_start(y[:, :], ob[:])


def _build_program():
    dt1 = BF if PHASE1_DT == "bf16" else F32
    nc = bacc.Bacc(num_devices=N_CORES)

    dlog_t = nc.declare_dram_parameter("dlog_t", [KC, B], dt1, isOutput=False)
    wp = nc.declare_dram_parameter("wp", [KC, E], dt1, isOutput=False)
    beta = nc.declare_dram_parameter("beta", [E], F32, isOutput=False)
    img_t = nc.declare_dram_parameter("img_t", [E, BL], BF, isOutput=False)
    capt_t = nc.declare_dram_parameter("capt_t", [E, BL], BF, isOutput=False)
    wm2 = nc.declare_dram_parameter("wm2", [3, E, 512], BF, isOutput=False)
    bias3 = nc.declare_dram_parameter("bias3", [3, 512], BF, isOutput=False)
    w2 = nc.declare_dram_parameter("w2", [RI, 128], BF, isOutput=False)
    w3 = nc.declare_dram_parameter("w3", [128, RI], BF, isOutput=False)
    y = nc.declare_dram_parameter("y", [BL, 128], F32, isOutput=True)
    io = (dlog_t, wp, beta, img_t, capt_t, wm2, bias3, w2, w3, y)

    with tile.TileContext(nc) as tc:
        with (
            tc.tile_pool(name="const", bufs=1) as const,
            tc.tile_pool(name="loads", bufs=3) as loads,
            tc.tile_pool(name="work", bufs=2) as work,
            tc.tile_pool(name="ps1", bufs=1, space="PSUM") as ps1,
            tc.tile_pool(name="dram", bufs=1, space="DRAM") as dram,
        ):
            _emit(nc, tc, const, loads, work, ps1, dram, io)

    nc.compile()
    return nc


def _host_prep(inputs):
    """Numpy-side sharding/layout prep. Returns per-core input maps."""
    img_emb = np.asarray(inputs["img_emb"], dtype=np.float32)
    capt_emb = np.asarray(inputs["capt_emb"], dtype=np.float32)
    dct = np.asarray(inputs["DCT_features"], dtype=np.float32).reshape(B, K)
    w_emb = np.asarray(inputs["W_emb"], dtype=np.float32)
    b_emb = np.asarray(inputs["b_emb"], dtype=np.float32)
    w_digit = np.asarray(inputs["W_digit"], dtype=np.float32)

    dlog = np.log(np.abs(dct) + 1e-12)
    mu = float(dlog.mean(dtype=np.float64))
    sigma = float(dlog.std(ddof=1, dtype=np.float64))
    s_w = w_emb.sum(axis=1, dtype=np.float64)
    beta = (b_emb - (mu / sigma) * s_w).astype(np.float32)

    np_dt1 = _BF16 if PHASE1_DT == "bf16" else np.float32
    dlog_T = np.ascontiguousarray(dlog.T).astype(np_dt1)  # [K, B]
    wp = np.ascontiguousarray(w_emb.T / sigma).astype(np_dt1)  # [K, E]

    wm2 = np.stack(
        [
            np.ascontiguousarray(
                np.asarray(inputs[f"W_{m}"], dtype=np.float32).transpose(2, 1, 0)
            ).reshape(E, 512)
            for m in ("img", "capt", "dct")
        ]
    ).astype(_BF16)  # [3, E, 512]
    bias3 = np.stack(
        [
            np.ascontiguousarray(
                np.asarray(inputs[f"b_{m}"], dtype=np.float32).T
            ).reshape(512)
            for m in ("img", "capt", "dct")
        ]
    ).astype(_BF16)  # [3, 512]
    w2 = (
        np.ascontiguousarray(w_digit.transpose(0, 3, 1, 2))
        .reshape(RI, 128)
        .astype(_BF16)
    )
    w3 = np.concatenate(
        [
            np.ascontiguousarray(w_digit[:, c].transpose(1, 0, 2)).reshape(OC, RI)
            for c in range(NCLS)
        ]
    ).astype(_BF16)  # [128, RI]
    img_T = np.ascontiguousarray(img_emb.T).astype(_BF16)  # [E, B]
    capt_T = np.ascontiguousarray(capt_emb.T).astype(_BF16)

    in_maps = []
    for c in range(N_CORES):
        in_maps.append(
            {
                "dlog_t": np.ascontiguousarray(dlog_T[c * KC : (c + 1) * KC]),
                "wp": np.ascontiguousarray(wp[c * KC : (c + 1) * KC]),
                "beta": beta,
                "img_t": np.ascontiguousarray(img_T[:, c * BL : (c + 1) * BL]),
                "capt_t": np.ascontiguousarray(capt_T[:, c * BL : (c + 1) * BL]),
                "wm2": wm2,
                "bias3": bias3,
                "w2": w2,
                "w3": w3,
            }
        )
    return in_maps


def kernel(**inputs) -> np.ndarray:
    if "nc" not in _CACHE:
        _CACHE["nc"] = _build_program()
    nc = _CACHE["nc"]
    in_maps = _host_prep(inputs)
    trace = bool(int(os.environ.get("CAPS_TRACE", "0")))
    res = run_bass_kernel_spmd(nc, in_maps, list(range(N_CORES)), trace=trace)
    _CACHE["last_result"] = res
    out = np.concatenate([r["y"] for r in res.results], axis=0).reshape(
        B, NCLS, OC
    )
    return np.ascontiguousarray(out)[:, :, :, None]


# revision 11
# speedup vs baseline: 1.3779x; 1.0998x over previous
"""CapsNet Trainium2 kernel: 8-core SPMD Bass/Tile implementation.

Strategy (v3):
  Phase 1 (contraction-parallel): the dominant op is
     dct_emb = relu(norm(log|DCT|) @ W_emb.T + b_emb),  [512,102400]x[102400,768]
  Each core owns a 12800-wide slice of the 102400 contraction dim
  (~128us of bf16 PE time — the roofline). log|x| is precomputed on
  host (needed there for the global mean/std anyway) and the affine
  normalization folds into the matmul epilogue. Phase-1 streaming loads
  own the Sync DMA queue; constant loads ride other engine queues.

  Phase 2 (batch-parallel): the per-core partial G^T[768,512] products
  are combined with a bf16 ReduceScatter over the batch dim (DRAM
  layout [8, 768, 64] so rank r receives exactly its 64 batch columns).
  Each core runs primary caps + dynamic routing for its own 64 rows;
  the two batch-mean agreement reductions are tiny [2,192] AllReduces.
  The host concatenates the per-core [64,128] outputs.

  Dynamic routing never materializes u_hat[B,192,2,64]:
     s_c   = (W2 * c)^T @ u           (contraction over (r,i)=1536)
     P_c   = W3_c^T @ v_c             (back-projected v)
     a_rc  = sum_b sum_i u * P_c      (agreement, mul+reduce)
  The elementwise digit squash uses s*|s|/(1+s^2) (the 1e-7 eps only
  matters at |s|~3e-4 where the output is ~1e-11 — far below
  tolerance), so routing needs no scalar-engine table switches.
"""

import os
import sys

import numpy as np

if "/opt/trn_rl_repo" not in sys.path:
    sys.path.insert(0, "/opt/trn_rl_repo")

import concourse.bass as bass  # noqa: E402
import concourse.mybir as mybir  # noqa: E402
import concourse.tile as tile  # noqa: E402
from concourse import bacc  # noqa: E402
from concourse.bass_utils import run_bass_kernel_spmd  # noqa: E402
from concourse.masks import make_identity  # noqa: E402

try:
    import ml_dtypes  # noqa: E402

    _BF16 = ml_dtypes.bfloat16
except Exception:  # pragma: no cover
    _BF16 = None

N_CORES = 8
B = 512  # global batch
BL = B // N_CORES  # 64 local batch rows per core
K, KC = 102400, 12800  # contraction dim, per-core slice
E = 768  # embedding
ET = E // 128  # e chunks
KT = KC // 128  # k tiles per core (100)
GROUP = 5  # k tiles per load DMA
RI = 1536  # (route, in_cap) flat = 192*8
RT = RI // 128  # 12 tiles
NCLS = 2
OC = 64  # out caps channels
F32 = mybir.dt.float32
BF = mybir.dt.bfloat16

PHASE1_DT = os.environ.get("CAPS_P1_DT", "bf16")
RS_DT = os.environ.get("CAPS_RS_DT", "bf16")
# bisection: 1=phase1 only, 2=+RS, 3=+prim/squash/u2, 5=full
STOP = int(os.environ.get("CAPS_STOP", "5"))

_CACHE = {}


def _emit(nc, tc, const, loads, work, dram, io):
    dt1 = BF if PHASE1_DT == "bf16" else F32
    dt_rs = BF if RS_DT == "bf16" else F32
    rg = [list(range(N_CORES))]
    dlog_t, wp, beta, img_t, capt_t, wm2, bias3, w2, w3, y = io

    def debug_out(fill=None):
        out_sb = work.tile([BL, 128], F32, tag="outsb", name="outsb")
        nc.vector.memset(out_sb[:], 0.0)
        if fill is not None:
            fill(out_sb)
        nc.sync.dma_start(y[:, :], out_sb[:])

    # ---------------- phase 1: big matmul (loads on Sync queue) --------
    psA_cm = tc.tile_pool(name="psA", bufs=1, space="PSUM")
    psA = psA_cm.__enter__()
    g_ps = [psA.tile([128, B], F32, tag=f"g{ec}", name=f"g{ec}") for ec in range(ET)]
    n_loads = KT // GROUP
    for li in range(n_loads):
        k0 = li * GROUP * 128
        dlog = loads.tile([128, GROUP, B], dt1, tag="dlog")
        nc.sync.dma_start(
            dlog[:],
            dlog_t[k0 : k0 + GROUP * 128, :].rearrange("(s p) b -> p s b", p=128),
        )
        w_tile = loads.tile([128, GROUP, E], dt1, tag="w")
        nc.sync.dma_start(
            w_tile[:],
            wp[k0 : k0 + GROUP * 128, :].rearrange("(s p) e -> p s e", p=128),
        )
        for s in range(GROUP):
            kt = li * GROUP + s
            for ec in range(ET):
                nc.tensor.matmul(
                    g_ps[ec][:],
                    w_tile[:, s, ec * 128 : (ec + 1) * 128],
                    dlog[:, s, :],
                    start=(kt == 0),
                    stop=(kt == KT - 1),
                )

    # ---------------- constants (DMA on vector/scalar/tensor queues) ----
    eps_sq = const.tile([128, 1], F32)
    nc.vector.memset(eps_sq[:], 1e-7)
    ident_bf = const.tile([128, 128], BF)
    make_identity(nc, ident_bf[:])
    ident_f = const.tile([128, 128], F32)
    make_identity(nc, ident_f[:])
    beta_sb = const.tile([128, ET], F32)
    nc.gpsimd.dma_start(beta_sb[:], beta[:].rearrange("(t p) -> p t", p=128))
    emb_sb = {}  # (m, et) -> [128, BL] bf16 tile (lhsT source for prim)
    for m, src in ((0, img_t), (1, capt_t)):
        for et in range(ET):
            t = const.tile([128, BL], BF, tag=f"emb{m}_{et}", name=f"emb{m}_{et}")
            nc.gpsimd.dma_start(t[:], src[et * 128 : (et + 1) * 128, :])
            emb_sb[(m, et)] = t
    wm2_sb = {}
    for m in range(3):
        for et in range(ET):
            t = const.tile([128, 512], BF, tag=f"wm2_{m}_{et}", name=f"wm2_{m}_{et}")
            nc.scalar.dma_start(t[:], wm2[m, et * 128 : (et + 1) * 128, :])
            wm2_sb[(m, et)] = t
    bias_sb = []
    for m in range(3):
        t = const.tile([1, 512], BF, tag=f"bias{m}", name=f"bias{m}")
        nc.scalar.dma_start(t[:], bias3[m : m + 1, :])
        bias_sb.append(t)
    ones1 = const.tile([1, BL], BF)
    nc.vector.memset(ones1[:], 1.0)
    w2_sb = []
    for t_ in range(RT):
        t = const.tile([128, 128], BF, tag=f"w2_{t_}", name=f"w2_{t_}")
        nc.gpsimd.dma_start(t[:], w2[t_ * 128 : (t_ + 1) * 128, :])
        w2_sb.append(t)
    w3_sb = []
    for c in range(NCLS):
        t = const.tile([OC, RI], BF, tag=f"w3_{c}", name=f"w3_{c}")
        nc.gpsimd.dma_start(t[:], w3[c * OC : (c + 1) * OC, :])
        w3_sb.append(t)

    # evacuate PSUM -> SBUF -> cc_in[8, 768, 64]; ReduceScatter over batch
    cc_in = dram.tile([N_CORES, E, BL], dt_rs)
    for ec in range(ET):
        g_sb = work.tile([128, B], dt_rs, tag="gsb", bufs=2)
        if ec % 2 == 0:
            nc.vector.tensor_copy(g_sb[:], g_ps[ec][:])
        else:
            nc.scalar.copy(g_sb[:], g_ps[ec][:])
        nc.sync.dma_start(
            cc_in[:, ec * 128 : (ec + 1) * 128, :].rearrange("r p b -> p r b"),
            g_sb[:].rearrange("p (r b) -> p r b", r=N_CORES),
        )
    psA_cm.__exit__(None, None, None)
    ps1_cm = tc.tile_pool(name="psB", bufs=1, space="PSUM")
    ps1 = ps1_cm.__enter__()
    if STOP == 1:
        debug_out()
        return

    rs_out = dram.tile([E, BL], dt_rs)
    nc.gpsimd.collective_compute(
        "ReduceScatter",
        mybir.AluOpType.add,
        replica_groups=rg,
        ins=[cc_in[:]],
        outs=[rs_out[:]],
    )
    if STOP == 2:
        debug_out(lambda o: nc.sync.dma_start(o[:, :64], rs_out[:BL, :]))
        return

    # ---------------- primary caps + squash + transpose to u2 ----------
    u2_all = const.tile([128, RT, BL], BF)  # [(r,i)-tile, t, b]

    def prim_chain(m):
        pm = ps1.tile([BL, 512], F32, tag="pp", bufs=2, name="pm")
        for et in range(ET):
            nc.tensor.matmul(
                pm[:],
                emb_sb[(m, et)][:],
                wm2_sb[(m, et)][:],
                start=(et == 0),
                stop=False,
            )
        nc.tensor.matmul(pm[:], ones1[:], bias_sb[m][:], start=False, stop=True)
        upre_c = work.tile([BL, 512], F32, tag="upre", bufs=3, name="upre")
        nc.vector.tensor_copy(upre_c[:], pm[:])
        # squash over i (groups of 8 in the free dim), 64 routes here
        sq8 = work.tile([BL, 512], F32, tag="sq8", bufs=2)
        nc.vector.tensor_mul(sq8[:], upre_c[:], upre_c[:])
        usq = work.tile([BL, 64], F32, tag="usq")
        nc.vector.tensor_reduce(
            usq[:],
            sq8[:].rearrange("p (r i) -> p r i", i=8),
            axis=mybir.AxisListType.X,
            op=mybir.AluOpType.add,
        )
        t1 = work.tile([BL, 64], F32, tag="fa")
        nc.scalar.activation(
            t1[:], usq[:], mybir.ActivationFunctionType.Sqrt, bias=eps_sq[:BL, :]
        )
        t2 = work.tile([BL, 64], F32, tag="fb")
        nc.vector.tensor_scalar_add(t2[:], usq[:], 1.0)
        t3 = work.tile([BL, 64], F32, tag="fc")
        nc.vector.tensor_mul(t3[:], t1[:], t2[:])
        t4 = work.tile([BL, 64], F32, tag="fd")
        nc.vector.reciprocal_approx_fast(t4[:], t3[:])
        t5 = work.tile([BL, 64], F32, tag="fe")
        nc.vector.tensor_mul(t5[:], t4[:], usq[:])
        usquash = work.tile([BL, 512], BF, tag="usquash", bufs=2)
        nc.vector.tensor_tensor(
            usquash[:].rearrange("p (r i) -> p r i", i=8),
            upre_c[:].rearrange("p (r i) -> p r i", i=8),
            t5[:].broadcast_to([BL, 64, 8]),
            op=mybir.AluOpType.mult,
        )
        for j in range(4):
            tp = ps1.tile([128, BL], BF, tag="pp", bufs=2, name="tp")
            nc.tensor.transpose(
                tp[:], usquash[:, j * 128 : (j + 1) * 128], ident_bf[:BL, :BL]
            )
            nc.vector.tensor_copy(u2_all[:, 4 * m + j, :], tp[:])

    # img/capt chains execute on the PE during the ReduceScatter
    prim_chain(0)
    prim_chain(1)

    # iter-0 s-matmul partial accumulation over img/capt tiles (RS shadow)
    s_ps = [
        ps1.tile([OC, BL], F32, tag=f"sps{c}", bufs=1, name=f"sps{c}")
        for c in range(NCLS)
    ]
    for c in range(NCLS):
        for t_ in range(8):
            nc.tensor.matmul(
                s_ps[c][:],
                w2_sb[t_][:, c * OC : (c + 1) * OC],
                u2_all[:, t_, :],
                start=(t_ == 0),
                stop=False,
            )

    # dct embedding tiles: relu(rs_out + beta) -> bf16 (vector, no tables)
    gp = work.tile([128, ET, BL], dt_rs, tag="gp")
    nc.sync.dma_start(gp[:], rs_out[:].rearrange("(t p) b -> p t b", p=128))
    for et in range(ET):
        t = const.tile([128, BL], BF, tag=f"emb2_{et}", name=f"emb2_{et}")
        nc.vector.tensor_scalar(
            t[:],
            gp[:, et, :],
            beta_sb[:, et : et + 1],
            0.0,
            op0=mybir.AluOpType.add,
            op1=mybir.AluOpType.max,
        )
        emb_sb[(2, et)] = t
    prim_chain(2)
    if STOP == 3:
        debug_out(
            lambda o: nc.vector.tensor_copy(
                o[:, :64], u2_all[:BL, 0, :]
            )
        )
        return

    # ---------------- dynamic routing (batch-sharded) -------------------
    a_d = [dram.tile([NCLS, 192], F32, name=f"a_d{i}") for i in range(2)]
    ar_a = [
        dram.tile([NCLS, 192], F32, addr_space="Shared", name=f"ar_a{i}")
        for i in range(2)
    ]
    c_dram = [dram.tile([192, NCLS], F32, name=f"c_dram{i}") for i in range(2)]
    b_cur = None  # [2,192] logits tile

    v_both = None
    for it in range(3):
        rnd = it - 1
        if it == 0:
            mset = w2_sb  # uniform c folded into evac scale 1/192
        else:
            # softmax(b_cur) over routes -> c_sm [2,192]
            mx = work.tile([NCLS, 1], F32, tag="smx", name="smx")
            nc.vector.tensor_reduce(
                mx[:], b_cur[:], axis=mybir.AxisListType.X, op=mybir.AluOpType.max
            )
            mxn = work.tile([NCLS, 1], F32, tag="smxn", name="smxn")
            nc.vector.tensor_scalar_mul(mxn[:], mx[:], -1.0)
            ex = work.tile([NCLS, 192], F32, tag="sex", name="sex")
            nc.scalar.activation(
                ex[:], b_cur[:], mybir.ActivationFunctionType.Exp, bias=mxn[:]
            )
            sm = work.tile([NCLS, 1], F32, tag="ssm", name="ssm")
            nc.vector.tensor_reduce(
                sm[:], ex[:], axis=mybir.AxisListType.X, op=mybir.AluOpType.add
            )
            rcp = work.tile([NCLS, 1], F32, tag="srcp", name="srcp")
            nc.vector.reciprocal(rcp[:], sm[:])
            c_sm = work.tile([NCLS, 192], F32, tag="scs", name="scs")
            nc.vector.tensor_scalar(
                c_sm[:], ex[:], rcp[:], None, op0=mybir.AluOpType.mult
            )
            # c [2,192] -> DRAM [192,2] -> broadcast-read c_exp [128, RT, 2]
            nc.sync.dma_start(c_dram[rnd][:].rearrange("r c -> c r"), c_sm[:])
            c_exp = work.tile([128, RT, NCLS], F32, tag="cexp", name="cexp")
            for t_ in range(RT):
                eng = (nc.sync, nc.scalar, nc.gpsimd)[t_ % 3]
                eng.dma_start(
                    c_exp[:, t_, :],
                    c_dram[rnd][16 * t_ : 16 * (t_ + 1), :]
                    .broadcast_to([16, NCLS, 8])
                    .rearrange("j c r -> j r c"),
                )
            # mset[t] = w2[t] * c  (both classes in one op; vector/gpsimd)
            mset = []
            for t_ in range(RT):
                msc = work.tile([128, 128], BF, tag="msc", bufs=12, name="msc")
                eng = nc.vector if t_ % 2 == 0 else nc.gpsimd
                eng.tensor_tensor(
                    msc[:].rearrange("p (c o) -> p c o", c=NCLS),
                    w2_sb[t_][:].rearrange("p (c o) -> p c o", c=NCLS),
                    c_exp[:, t_, :].broadcast_to([128, NCLS, OC]),
                    op=mybir.AluOpType.mult,
                )
                mset.append(msc)

        # s per class
        if it > 0:
            s_ps = [
                ps1.tile([OC, BL], F32, tag=f"sps{c}", bufs=1, name=f"sps{c}")
                for c in range(NCLS)
            ]
        v_both = []
        for c in range(NCLS):
            t_start = 8 if it == 0 else 0
            for t_ in range(t_start, RT):
                lh = mset[t_][:, c * OC : (c + 1) * OC]
                nc.tensor.matmul(
                    s_ps[c][:],
                    lh,
                    u2_all[:, t_, :],
                    start=(it > 0 and t_ == 0),
                    stop=(t_ == RT - 1),
                )
            s_sb = work.tile([OC, BL], F32, tag=f"ssb{c}", name=f"ssb{c}")
            nc.vector.tensor_scalar_mul(
                s_sb[:], s_ps[c][:], (1.0 / 192.0) if it == 0 else 1.0
            )
            # elementwise digit squash v = s*|s|/(1+s^2) on [OC, BL]
            sq = work.tile([OC, BL], F32, tag=f"dsq{c}", name=f"dsq{c}")
            nc.vector.tensor_mul(sq[:], s_sb[:], s_sb[:])
            den = work.tile([OC, BL], F32, tag=f"dd2{c}", name=f"dd2{c}")
            nc.vector.tensor_scalar_add(den[:], sq[:], 1.0)
            rec = work.tile([OC, BL], F32, tag=f"dd4{c}", name=f"dd4{c}")
            nc.vector.reciprocal_approx_fast(rec[:], den[:])
            ns = work.tile([OC, BL], F32, tag=f"dn{c}", name=f"dn{c}")
            nc.vector.tensor_scalar_mul(ns[:], s_sb[:], -1.0)
            sab = work.tile([OC, BL], F32, tag=f"dd1{c}", name=f"dd1{c}")
            nc.vector.tensor_tensor(
                sab[:], s_sb[:], ns[:], op=mybir.AluOpType.max
            )
            num = work.tile([OC, BL], F32, tag=f"dd3{c}", name=f"dd3{c}")
            nc.vector.tensor_mul(num[:], sab[:], s_sb[:])
            vv = work.tile(
                [OC, BL],
                BF if it < 2 else F32,
                tag=f"vb{c}{'f' if it == 2 else ''}",
                name=f"vb{c}",
                bufs=2,
            )
            nc.vector.tensor_mul(vv[:], num[:], rec[:])
            v_both.append(vv)

        if it < 2:
            # agreement: abar[r,c] = sum_b sum_i u2 * (W3_c^T @ v_c)
            for c in range(NCLS):
                pc_all = ps1.tile(
                    [128, RT * BL], F32, tag="pca", bufs=1, name="pca"
                )
                for t_ in range(RT):
                    nc.tensor.matmul(
                        pc_all[:, t_ * BL : (t_ + 1) * BL],
                        w3_sb[c][:, t_ * 128 : (t_ + 1) * 128],
                        v_both[c][:],
                        start=True,
                        stop=True,
                    )
                prod = work.tile([128, RT * BL], BF, tag="prod", name="prod")
                nc.vector.tensor_mul(
                    prod[:], u2_all[:].rearrange("p t b -> p (t b)"), pc_all[:]
                )
                dcat = work.tile([128, RT], F32, tag=f"dcat{c}", name=f"dcat{c}")
                nc.vector.tensor_reduce(
                    dcat[:],
                    prod[:].rearrange("p (t b) -> p t b", t=RT),
                    axis=mybir.AxisListType.X,
                    op=mybir.AluOpType.add,
                )
                dtp = ps1.tile([RT, 128], F32, tag="pp", bufs=2, name="dtp")
                nc.tensor.transpose(dtp[:], dcat[:], ident_f[:])
                abar = work.tile([RT, 16], F32, tag=f"abar{c}", name=f"abar{c}")
                nc.vector.tensor_reduce(
                    abar[:],
                    dtp[:].rearrange("p (r i) -> p r i", i=8),
                    axis=mybir.AxisListType.X,
                    op=mybir.AluOpType.add,
                )
                nc.sync.dma_start(
                    a_d[it][c, :].rearrange("(t j) -> t j", t=RT), abar[:]
                )
            nc.gpsimd.collective_compute(
                "AllReduce",
                mybir.AluOpType.add,
                replica_groups=rg,
                ins=[a_d[it][:]],
                outs=[ar_a[it][:]],
            )
            ld = work.tile([NCLS, 192], F32, tag=f"arld{it}", name=f"arld{it}")
            nc.sync.dma_start(ld[:], ar_a[it][:])
            b_new = work.tile([NCLS, 192], F32, tag=f"bcur{it}", name=f"bcur{it}")
            if it == 0:
                nc.vector.tensor_scalar_mul(b_new[:], ld[:], 1.0 / B)
            else:
                scaled = work.tile([NCLS, 192], F32, tag="arsc", name="arsc")
                nc.vector.tensor_scalar_mul(scaled[:], ld[:], 1.0 / B)
                nc.vector.tensor_add(b_new[:], b_cur[:], scaled[:])
            b_cur = b_new

    # final output: y[b, (c,o)] via PE transposes of v_c
    ob = work.tile([BL, 128], F32, tag="ob", bufs=2, name="ob")
    for c in range(NCLS):
        vt = ps1.tile([BL, OC], F32, tag="pp", bufs=2, name="vt")
        nc.tensor.transpose(vt[:], v_both[c][:], ident_f[:OC, :OC])
        nc.vector.tensor_copy(ob[:, c * OC : (c + 1) * OC], vt[:])
    nc.sync.dma_start(y[:, :], ob[:])
    ps1_cm.__exit__(None, None, None)


def _build_program():
    dt1 = BF if PHASE1_DT == "bf16" else F32
    nc = bacc.Bacc(num_devices=N_CORES)

    dlog_t = nc.declare_dram_parameter("dlog_t", [KC, B], dt1, isOutput=False)
    wp = nc.declare_dram_parameter("wp", [KC, E], dt1, isOutput=False)
    beta = nc.declare_dram_parameter("beta", [E], F32, isOutput=False)
    img_t = nc.declare_dram_parameter("img_t", [E, BL], BF, isOutput=False)
    capt_t = nc.declare_dram_parameter("capt_t", [E, BL], BF, isOutput=False)
    wm2 = nc.declare_dram_parameter("wm2", [3, E, 512], BF, isOutput=False)
    bias3 = nc.declare_dram_parameter("bias3", [3, 512], BF, isOutput=False)
    w2 = nc.declare_dram_parameter("w2", [RI, 128], BF, isOutput=False)
    w3 = nc.declare_dram_parameter("w3", [128, RI], BF, isOutput=False)
    y = nc.declare_dram_parameter("y", [BL, 128], F32, isOutput=True)
    io = (dlog_t, wp, beta, img_t, capt_t, wm2, bias3, w2, w3, y)

    with tile.TileContext(nc) as tc:
        with (
            tc.tile_pool(name="const", bufs=1) as const,
            tc.tile_pool(name="loads", bufs=3) as loads,
            tc.tile_pool(name="work", bufs=2) as work,
            tc.tile_pool(name="dram", bufs=1, space="DRAM") as dram,
        ):
            _emit(nc, tc, const, loads, work, dram, io)

    nc.compile()
    return nc


def _host_prep(inputs):
    """Numpy-side sharding/layout prep. Returns per-core input maps."""
    img_emb = np.asarray(inputs["img_emb"], dtype=np.float32)
    capt_emb = np.asarray(inputs["capt_emb"], dtype=np.float32)
    dct = np.asarray(inputs["DCT_features"], dtype=np.float32).reshape(B, K)
    w_emb = np.asarray(inputs["W_emb"], dtype=np.float32)
    b_emb = np.asarray(inputs["b_emb"], dtype=np.float32)
    w_digit = np.asarray(inputs["W_digit"], dtype=np.float32)

    dlog = np.log(np.abs(dct) + 1e-12)
    mu = float(dlog.mean(dtype=np.float64))
    sigma = float(dlog.std(ddof=1, dtype=np.float64))
    s_w = w_emb.sum(axis=1, dtype=np.float64)
    beta = (b_emb - (mu / sigma) * s_w).astype(np.float32)

    np_dt1 = _BF16 if PHASE1_DT == "bf16" else np.float32
    dlog_T = np.ascontiguousarray(dlog.T).astype(np_dt1)  # [K, B]
    wp = np.ascontiguousarray(w_emb.T / sigma).astype(np_dt1)  # [K, E]

    wm2 = np.stack(
        [
            np.ascontiguousarray(
                np.asarray(inputs[f"W_{m}"], dtype=np.float32).transpose(2, 1, 0)
            ).reshape(E, 512)
            for m in ("img", "capt", "dct")
        ]
    ).astype(_BF16)  # [3, E, 512]
    bias3 = np.stack(
        [
            np.ascontiguousarray(
                np.asarray(inputs[f"b_{m}"], dtype=np.float32).T
            ).reshape(512)
            for m in ("img", "capt", "dct")
        ]
    ).astype(_BF16)  # [3, 512]
    w2 = (
        np.ascontiguousarray(w_digit.transpose(0, 3, 1, 2))
        .reshape(RI, 128)
        .astype(_BF16)
    )
    w3 = np.concatenate(
        [
            np.ascontiguousarray(w_digit[:, c].transpose(1, 0, 2)).reshape(OC, RI)
            for c in range(NCLS)
        ]
    ).astype(_BF16)  # [128, RI]
    img_T = np.ascontiguousarray(img_emb.T).astype(_BF16)  # [E, B]
    capt_T = np.ascontiguousarray(capt_emb.T).astype(_BF16)

    in_maps = []
    for c in range(N_CORES):
        in_maps.append(
            {
                "dlog_t": np.ascontiguousarray(dlog_T[c * KC : (c + 1) * KC]),
                "wp": np.ascontiguousarray(wp[c * KC : (c + 1) * KC]),
                "beta": beta,
                "img_t": np.ascontiguousarray(img_T[:, c * BL : (c + 1) * BL]),
                "capt_t": np.ascontiguousarray(capt_T[:, c * BL : (c + 1) * BL]),
                "wm2": wm2,
                "bias3": bias3,
                "w2": w2,
                "w3": w3,
            }
        )
    return in_maps


def kernel(**inputs) -> np.ndarray:
    if "nc" not in _CACHE:
        _CACHE["nc"] = _build_program()
    nc = _CACHE["nc"]
    in_maps = _host_prep(inputs)
    trace = bool(int(os.environ.get("CAPS_TRACE", "0")))
    res = run_bass_kernel_spmd(nc, in_maps, list(range(N_CORES)), trace=trace)
    _CACHE["last_result"] = res
    out = np.concatenate([r["y"] for r in res.results], axis=0).reshape(
        B, NCLS, OC
    )
    return np.ascontiguousarray(out)[:, :, :, None]
